# revision 1
# baseline (speedup 1.0000x reference)
"""Trainium2 Bass kernel for nn_EnhancedRecurrentGCN (K=1 DConv DCRNN stack).

Math (h0 == 0 collapses each DCRNN cell; the r-gate is multiplied by zero):
    h1 = relu(sigmoid(-x@W1z) * tanh(x@W1h))     [per node]
    h2 = relu(sigmoid(-h1@W2z) * tanh(h1@W2h))
    y  = relu(h2@W3 + b3) @ W4 + b4
with W1z = (w_z1[0,0]+w_z1[1,0])[:256] etc.  edge_index/edge_weight unused.

Design:
 - Pure data parallelism: x padded to 102400 rows, 12800 nodes/core, shipped
   pre-transposed and cast to fp16 (xt = x_shard.T) so the contraction dim is
   the SBUF partition dim.  All device tensors are feature-major.
 - fp16 matmul operands (fp32 PSUM accumulate): full PE rate, ~9e-4 rel err.
 - relu(s*u) = s*max(u,0) fused into one DVE scalar_tensor_tensor per layer.
 - Layer-2/3/4 outputs partition-packed via zero-embedded weight copies
   accumulating in psum (tile_position col/row offsets are rejected for
   these dtypes): L2 packs 2 subtiles onto 64+64 partitions; L3 packs 8
   subtiles onto 16*8 partitions; L4 is ONE matmul per 8 subtiles.
 - Layer-2 sigma fusion: tanh(b) = 2*sigmoid(2b)-1 with the -2 baked into
   the zero-embedded W2h copies, so sigmoid+tanh is one ACT call per macro.
 - Two-stage software pipelining (macro m+1's L1 matmuls emitted before
   macro m's tail) to overlap the in-order engine queues.
 - Split packed weight DMA + ramped x chunks to shorten the prologue;
   per-group output DMAs to shorten the epilogue.
"""

import os
import sys

if "/opt/trn_rl_repo" not in sys.path:
    sys.path.insert(0, "/opt/trn_rl_repo")

from contextlib import ExitStack

import numpy as np
import ml_dtypes

import concourse.mybir as mybir
import concourse.tile as tile
from concourse import bacc
from concourse.bass_utils import run_bass_kernel_spmd

N_CORES = 8
PAD_NODES = 102400
SHARD = PAD_NODES // N_CORES  # 12800
TN = 512
CHUNK = 2560
GROUP = 8

F32 = mybir.dt.float32
F32R = mybir.dt.float32r
BF16 = mybir.dt.bfloat16
AF = mybir.ActivationFunctionType
OP = mybir.AluOpType

FP16 = mybir.dt.float16
_KMMDT = os.environ.get("KMMDT", "fp16")
MMDT = {"bf16": BF16, "fp16": FP16, "f32r": F32R}[_KMMDT]
NPDT = {"bf16": ml_dtypes.bfloat16, "fp16": np.float16, "f32r": np.float32}[_KMMDT]
# fp16 matmul operands: full PE rate (1 cycle/col vs 2 for fp32r), half the
# input DMA, and ~7e-4 relative error (all values here are << fp16 range).


def build_nc(shard=SHARD, mmdt=None):
    if mmdt is None:
        mmdt = MMDT
    assert shard % TN == 0
    n_sub = shard // TN
    groups = [list(range(g, min(g + GROUP, n_sub)))
              for g in range(0, n_sub, GROUP)]
    n_grp = len(groups)
    chunks = []
    c = 0
    ramp = [512, 512, 1024, 2048]
    while c < shard:
        w = min(ramp[len(chunks)] if len(chunks) < len(ramp) else CHUNK,
                shard - c)
        chunks.append((c, w))
        c += w

    nc = bacc.Bacc(None)

    WCOLS = 2048 + GROUP  # 16 x 128-col weight slabs + the w4 embed
    xt = nc.declare_dram_parameter("xt", [256, shard], mmdt, isOutput=False)
    wpackA = nc.declare_dram_parameter("wpackA", [128, 512], mmdt, isOutput=False)
    wpackB = nc.declare_dram_parameter("wpackB", [128, WCOLS - 512], mmdt,
                                       isOutput=False)
    bpack = nc.declare_dram_parameter("bpack", [128, 6], F32, isOutput=False)
    out = nc.declare_dram_parameter("out", [GROUP, TN * n_grp], F32, isOutput=True)

    with ExitStack() as ctx:
        tc = ctx.enter_context(tile.TileContext(nc, pool_alloc_mode="queue"))
        wp = ctx.enter_context(tc.tile_pool(name="weights", bufs=1))
        xp = ctx.enter_context(tc.tile_pool(name="x", bufs=4))
        ap = ctx.enter_context(tc.tile_pool(name="acts", bufs=6))
        ob = ctx.enter_context(tc.tile_pool(name="outbuf", bufs=1))
        # PSUM: zpre 2 + tpre 2 + z2p 1 + t2p 1 + h3p 1 + opre 1 = 8 banks
        pz1 = ctx.enter_context(tc.tile_pool(name="pz1", bufs=1, space="PSUM"))
        pt1 = ctx.enter_context(tc.tile_pool(name="pt1", bufs=1, space="PSUM"))
        pzt2 = ctx.enter_context(tc.tile_pool(name="pzt2", bufs=1, space="PSUM"))
        ph3 = ctx.enter_context(tc.tile_pool(name="ph3", bufs=1, space="PSUM"))
        po = ctx.enter_context(tc.tile_pool(name="po", bufs=1, space="PSUM"))

        wpack_sb = wp.tile([128, WCOLS], mmdt, name="wpack_sb")
        nc.sync.dma_start(wpack_sb[:, 0:512], wpackA[:])
        nc.sync.dma_start(wpack_sb[:, 512:WCOLS], wpackB[:])
        bpack_sb = wp.tile([128, 6], F32, name="bpack_sb")
        nc.sync.dma_start(bpack_sb[:], bpack[:])

        def wslab(k):
            return wpack_sb[:, 128 * k:128 * (k + 1)]

        w1z_sb = [wslab(0), wslab(1)]
        w1h_sb = [wslab(2), wslab(3)]
        w2ze_sb = [wslab(4), wslab(5)]
        w2he_sb = [wslab(6), wslab(7)]
        w3e_sb = [wslab(8 + j) for j in range(GROUP)]
        w4e_sb = wpack_sb[:, 2048:2048 + GROUP]
        bias_sb = {nm: bpack_sb[:, k:k + 1]
                   for k, nm in enumerate(["nbz1", "bh1", "nbz2p", "bh2p",
                                           "b3sp", "b4bc"])}

        out_sb = ob.tile([GROUP, TN * n_grp], F32)

        x_tiles = {}

        def ensure_chunk(ci):
            if ci in x_tiles or ci >= len(chunks):
                return
            c0, cw = chunks[ci]
            xa = xp.tile([128, cw], mmdt, tag="xa", name=f"xa{ci}")
            xb = xp.tile([128, cw], mmdt, tag="xb", name=f"xb{ci}")
            nc.sync.dma_start(xa[:], xt[0:128, c0:c0 + cw])
            nc.sync.dma_start(xb[:], xt[128:256, c0:c0 + cw])
            x_tiles[ci] = (xa, xb)

        def x_slice(s):
            col = s * TN
            ci = next(k for k, (c0, cw) in enumerate(chunks)
                      if c0 <= col < c0 + cw)
            ensure_chunk(ci)
            ensure_chunk(ci + 1)
            off = col - chunks[ci][0]
            return x_tiles[ci][0], x_tiles[ci][1], slice(off, off + TN)

        macros = [(2 * m, 2) for m in range(n_sub // 2)]
        if n_sub % 2:
            macros.append((n_sub - 1, 1))

        h3p_cur = [None]

        def stage_a(s0, nsub):
            """Layer-1 matmuls for macro (s0, nsub) -> (zpre, tpre)."""
            mw = nsub * TN
            zpre = pz1.tile([128, mw], F32, tag="zpre", name=f"zpre{s0}")
            tpre = pt1.tile([128, mw], F32, tag="tpre", name=f"tpre{s0}")
            for i in range(nsub):
                xa, xb, sl = x_slice(s0 + i)
                d = slice(i * TN, (i + 1) * TN)
                nc.tensor.matmul(zpre[:, d], w1z_sb[0], xa[:, sl],
                                 start=True, stop=False, skip_group_check=True)
                nc.tensor.matmul(zpre[:, d], w1z_sb[1], xb[:, sl],
                                 start=False, stop=True, skip_group_check=True)
            for i in range(nsub):
                xa, xb, sl = x_slice(s0 + i)
                d = slice(i * TN, (i + 1) * TN)
                nc.tensor.matmul(tpre[:, d], w1h_sb[0], xa[:, sl],
                                 start=True, stop=False, skip_group_check=True)
                nc.tensor.matmul(tpre[:, d], w1h_sb[1], xb[:, sl],
                                 start=False, stop=True, skip_group_check=True)
            return zpre, tpre

        def stage_b(s0, nsub, zpre, tpre):
            """ACT/DVE + layers 2-4 for macro (s0, nsub)."""
            mw = nsub * TN
            s1t = ap.tile([128, mw], F32, tag="s1t", name=f"s1t{s0}")
            nc.scalar.activation(s1t[:], zpre[:], AF.Sigmoid,
                                 bias=bias_sb["nbz1"], scale=-1.0)
            u1t = ap.tile([128, mw], F32, tag="u1t", name=f"u1t{s0}")
            nc.scalar.activation(u1t[:], tpre[:], AF.Tanh,
                                 bias=bias_sb["bh1"], scale=1.0)
            h1t = ap.tile([128, mw], mmdt, tag="h1t", name=f"h1t{s0}")
            nc.vector.scalar_tensor_tensor(h1t[:], u1t[:], 0.0, s1t[:],
                                           op0=OP.max, op1=OP.mult)

            # zt2 = [ a2 | -2*b2 ]; one sigmoid(-x) gives [ s2 | sig(2b2) ]
            # and tanh(b2) = 2*sig(2b2) - 1  (w2he embeds carry the -2).
            zt2 = pzt2.tile([128, 2 * TN], F32, tag="zt2", name=f"zt2{s0}")
            for half, wsb in enumerate([w2ze_sb, w2he_sb]):
                d = slice(half * TN, (half + 1) * TN)
                for i in range(nsub):
                    nc.tensor.matmul(zt2[:, d], wsb[i],
                                     h1t[:, i * TN:(i + 1) * TN],
                                     start=(i == 0), stop=(i == nsub - 1),
                                     skip_group_check=True)
            sv2 = ap.tile([128, 2 * TN], F32, tag="sv2", name=f"sv2{s0}")
            nc.scalar.activation(sv2[:], zt2[:], AF.Sigmoid, scale=-1.0)
            w2i = ap.tile([128, TN], F32, tag="w2i", name=f"w2i{s0}")
            nc.vector.scalar_tensor_tensor(w2i[:], sv2[:, TN:2 * TN], 0.5,
                                           sv2[:, 0:TN], op0=OP.max,
                                           op1=OP.mult)
            h2t = ap.tile([128, TN], mmdt, tag="h2t", name=f"h2t{s0}")
            nc.vector.scalar_tensor_tensor(h2t[:], w2i[:], 2.0, sv2[:, 0:TN],
                                           op0=OP.mult, op1=OP.subtract)

            for i in range(nsub):
                s = s0 + i
                g = s // GROUP
                j = s - groups[g][0]
                if j == 0:
                    h3p_cur[0] = ph3.tile([128, TN], F32, tag="h3p",
                                          name=f"h3p{g}")
                last = (j == len(groups[g]) - 1)
                nc.tensor.matmul(h3p_cur[0][:], w3e_sb[j], h2t[:],
                                 start=(j == 0), stop=last,
                                 skip_group_check=True)
                if last:
                    gs = len(groups[g])
                    h3s = ap.tile([128, TN], mmdt, tag="h3s", name=f"h3s{g}")
                    nc.vector.tensor_scalar(
                        h3s[0:16 * gs, :], h3p_cur[0][0:16 * gs, :],
                        bpack_sb[0:16 * gs, 4:5], 0.0,
                        op0=OP.add, op1=OP.max)
                    opre = po.tile([GROUP, TN], F32, tag="opre", name=f"op{g}")
                    nc.tensor.matmul(opre[0:gs, :], wpack_sb[0:16 * gs, 2048:2048 + gs],
                                     h3s[0:16 * gs, :], start=True, stop=True,
                                     skip_group_check=True)
                    nc.vector.tensor_scalar(
                        out_sb[0:gs, g * TN:(g + 1) * TN], opre[0:gs, :],
                        bpack_sb[0:gs, 5:6], None, op0=OP.add)
                    nc.sync.dma_start(out[0:gs, g * TN:(g + 1) * TN],
                                      out_sb[0:gs, g * TN:(g + 1) * TN])

        # two-stage software pipeline over macros
        pend = None
        for s0, nsub in macros:
            zp = stage_a(s0, nsub)
            if pend is not None:
                stage_b(*pend)
            pend = (s0, nsub, *zp)
        stage_b(*pend)

    nc.compile()
    return nc


_NC_CACHE = {}


def _get_nc(shard=SHARD):
    if shard not in _NC_CACHE:
        _NC_CACHE[shard] = build_nc(shard)
    return _NC_CACHE[shard]


def make_in_maps(x, w_z1, b_z1, w_r1, b_r1, w_h1, b_h1,
                 w_z2, b_z2, w_r2, b_r2, w_h2, b_h2,
                 w_lin1, b_lin1, w_lin2, b_lin2,
                 n_cores=N_CORES, shard=SHARD):
    f = np.float32
    for b in (b_z2, b_h2):
        assert not np.any(np.asarray(b)), \
            "sigma-fused layer 2 assumes zero gate biases (spec: fill=zeros)"
    w1z = np.asarray((np.asarray(w_z1)[0, 0] + np.asarray(w_z1)[1, 0])[:256], f)
    w1h = np.asarray((np.asarray(w_h1)[0, 0] + np.asarray(w_h1)[1, 0])[:256], f)
    w2z = np.asarray((np.asarray(w_z2)[0, 0] + np.asarray(w_z2)[1, 0])[:128], f)
    w2h = np.asarray((np.asarray(w_h2)[0, 0] + np.asarray(w_h2)[1, 0])[:128], f)
    w3 = np.asarray(w_lin1, f)
    w4 = np.asarray(w_lin2, f)

    wp = np.zeros((128, 2048 + GROUP), f)
    wp[:, 0:128] = w1z[0:128]
    wp[:, 128:256] = w1z[128:256]
    wp[:, 256:384] = w1h[0:128]
    wp[:, 384:512] = w1h[128:256]
    for v in range(2):
        wp[:, 512 + 128 * v + 64 * v:512 + 128 * v + 64 * v + 64] = w2z
        wp[:, 768 + 128 * v + 64 * v:768 + 128 * v + 64 * v + 64] = -2.0 * w2h
    for j in range(GROUP):
        h = 64 * (j % 2)
        wp[h:h + 64, 1024 + 128 * j + 16 * j:1024 + 128 * j + 16 * j + 16] = w3
    for j in range(GROUP):
        wp[16 * j:16 * j + 16, 2048 + j] = w4[:, 0]

    bp = np.zeros((128, 6), f)
    bp[:, 0] = -np.asarray(b_z1, f)
    bp[:, 1] = np.asarray(b_h1, f)
    bp[:, 2] = -np.tile(np.asarray(b_z2, f), 2)
    bp[:, 3] = np.tile(np.asarray(b_h2, f), 2)
    bp[:, 4] = np.tile(np.asarray(b_lin1, f), GROUP)
    bp[0:GROUP, 5] = np.asarray(b_lin2, f).reshape(-1)[0]
    wpn = wp.astype(NPDT)
    common = {
        "wpackA": np.ascontiguousarray(wpn[:, 0:512]),
        "wpackB": np.ascontiguousarray(wpn[:, 512:]),
        "bpack": bp,
    }
    x = np.asarray(x, f)
    n = x.shape[0]
    pad = n_cores * shard
    xpad = np.zeros((pad, 256), f)
    xpad[:n] = x
    shards = xpad.reshape(n_cores, shard, 256)
    return [dict(common, xt=np.ascontiguousarray(shards[i].T).astype(NPDT))
            for i in range(n_cores)]


def unscramble(res, n_cores=N_CORES, shard=SHARD):
    n_sub = shard // TN
    full = np.empty(n_cores * shard, np.float32)
    for i in range(n_cores):
        o = res[i]
        for g in range((n_sub + GROUP - 1) // GROUP):
            gs = min(GROUP, n_sub - g * GROUP)
            for j in range(gs):
                s = g * GROUP + j
                full[i * shard + s * TN:i * shard + (s + 1) * TN] = \
                    o[j, g * TN:(g + 1) * TN]
    return full


def kernel(x, edge_index=None, edge_weight=None,
           w_z1=None, b_z1=None, w_r1=None, b_r1=None, w_h1=None, b_h1=None,
           w_z2=None, b_z2=None, w_r2=None, b_r2=None, w_h2=None, b_h2=None,
           w_lin1=None, b_lin1=None, w_lin2=None, b_lin2=None):
    in_maps = make_in_maps(x, w_z1, b_z1, w_r1, b_r1, w_h1, b_h1,
                           w_z2, b_z2, w_r2, b_r2, w_h2, b_h2,
                           w_lin1, b_lin1, w_lin2, b_lin2)
    nc = _get_nc()
    res = run_bass_kernel_spmd(nc, in_maps, list(range(N_CORES))).results
    n = np.asarray(x).shape[0]
    full = unscramble([res[i]["out"] for i in range(N_CORES)])
    return np.ascontiguousarray(full[:n].reshape(n, 1).astype(np.float32))



# revision 2
# speedup vs baseline: 1.0192x; 1.0192x over previous
"""Trainium2 Bass kernel for nn_EnhancedRecurrentGCN (K=1 DConv DCRNN stack).

Math (h0 == 0 collapses each DCRNN cell; the r-gate is multiplied by zero):
    h1 = relu(sigmoid(-x@W1z) * tanh(x@W1h))     [per node]
    h2 = relu(sigmoid(-h1@W2z) * tanh(h1@W2h))
    y  = relu(h2@W3 + b3) @ W4 + b4

Design (v5, shipped):
 - 12800 nodes/core (padded), TN=512, 25 subtiles.
 - Sigmoid-only activations: tanh(b) = 2*sigmoid(2b)-1, -2 baked into W*h
   slabs; ONE sigmoid ACTIVATE per L1 subtile ([128,1024] psum: z cols
   0:512, -2b cols 512:1024) and one per L2 macro.
 - relu(s*tanh(b)) = s*relu(2v-1) = 2 * (max(v,.5)-.5) * s with the 2
   folded into the next layer's weights.  Per gate-pair: one 4x-mode
   tensor_scalar + one 2x-mode tensor_tensor (bf16/fp16), no STT ops.
 - L2 via [128,64] slabs writing at psum partition base 0/64 (macro pair
   packing) - no zero-embedded slabs, cheaper LDWEIGHTS.
 - x shipped as [128, 2*12800] fp16 (chunk-interleaved halves) so each
   chunk is ONE DMA; first chunk issued before the weight DMAs.
"""

import os
import sys

if "/opt/trn_rl_repo" not in sys.path:
    sys.path.insert(0, "/opt/trn_rl_repo")

from contextlib import ExitStack

import numpy as np

import concourse.mybir as mybir
import concourse.tile as tile
from concourse import bacc
from concourse.bass_utils import run_bass_kernel_spmd

N_CORES = 8
SHARD = 12800
TN = 512
GROUP = 8
N_SUB = SHARD // TN           # 25
N_GRP = (N_SUB + GROUP - 1) // GROUP  # 4 (8+8+8+1)

F32 = mybir.dt.float32
FP16 = mybir.dt.float16
BF16 = mybir.dt.bfloat16
AF = mybir.ActivationFunctionType
OP = mybir.AluOpType

# sigma-output dtypes: bf16 for layer 1 (speed; error averages out through
# the L2 contraction), fp16 for layer 2 (its quantization hits y directly:
# bf16 here costs 1.7e-2 final error vs 6.2e-3 with fp16).
SV1DT = _D = BF16
SV2DT = FP16

# weight pack columns: L1 4x128 | L2 2x64 | L3 4x128 | L4 8
W2OFF = 512
W3OFF = 512 + 128
W4OFF = W3OFF + 512
WCOLS = W4OFF + 8
CHUNKS = [512, 512, 1024, 2048, 2560, 3072, 3072]  # = 12800


def _gs(g):
    return min(GROUP, N_SUB - g * GROUP)


def build_nc():
    nc = bacc.Bacc(None)

    xt = nc.declare_dram_parameter("xt", [128, 2 * SHARD], FP16, isOutput=False)
    wpackA = nc.declare_dram_parameter("wpackA", [128, 512], FP16, isOutput=False)
    wpackB = nc.declare_dram_parameter("wpackB", [128, WCOLS - 512], FP16,
                                       isOutput=False)
    bpack = nc.declare_dram_parameter("bpack", [128, 2], F32, isOutput=False)
    out = nc.declare_dram_parameter("out", [GROUP, TN * N_GRP], F32, isOutput=True)

    with ExitStack() as ctx:
        tc = ctx.enter_context(tile.TileContext(nc, pool_alloc_mode="queue"))
        wp = ctx.enter_context(tc.tile_pool(name="weights", bufs=1))
        xp = ctx.enter_context(tc.tile_pool(name="x", bufs=4))
        sv1p = ctx.enter_context(tc.tile_pool(name="sv1", bufs=3))
        u1p = ctx.enter_context(tc.tile_pool(name="u1", bufs=2))
        h1p = ctx.enter_context(tc.tile_pool(name="h1t", bufs=3))
        sv2p = ctx.enter_context(tc.tile_pool(name="sv2", bufs=2))
        u2p = ctx.enter_context(tc.tile_pool(name="u2", bufs=2))
        h2p = ctx.enter_context(tc.tile_pool(name="h2t", bufs=3))
        h3sp = ctx.enter_context(tc.tile_pool(name="h3s", bufs=2))
        ob = ctx.enter_context(tc.tile_pool(name="outbuf", bufs=1))
        # PSUM: l1p 2x2 banks + zt2 2 + h3p 1 + opre 1 = 8 banks
        pl1 = ctx.enter_context(tc.tile_pool(name="pl1", bufs=2, space="PSUM"))
        pzt = ctx.enter_context(tc.tile_pool(name="pzt", bufs=1, space="PSUM"))
        ph3 = ctx.enter_context(tc.tile_pool(name="ph3", bufs=1, space="PSUM"))
        po = ctx.enter_context(tc.tile_pool(name="po", bufs=1, space="PSUM"))

        x_tiles = {}
        chunk_off = []
        c = 0
        for w in CHUNKS:
            chunk_off.append((c, w))
            c += w

        def ensure_chunk(ci):
            if ci in x_tiles or ci >= len(chunk_off):
                return
            c0, cw = chunk_off[ci]
            xab = xp.tile([128, 2 * cw], FP16, tag="xab", name=f"xab{ci}")
            nc.sync.dma_start(xab[:], xt[:, 2 * c0:2 * c0 + 2 * cw])
            x_tiles[ci] = xab

        ensure_chunk(0)  # x before weights: shortest path to first matmul

        # PE warm-up on junk data: keeps the HAM activity window hot while
        # the first x chunk is in flight; psum bank is overwritten later.
        junk = wp.tile([128, 512], FP16, name="junk")
        nc.gpsimd.memset(junk[:], 0.0)
        warm_ps = ph3.tile([128, 512], F32, tag="h3p", name="warm")
        for _ in range(10):
            nc.tensor.matmul(warm_ps[:, 0:TN], junk[:, 0:128], junk[:, 0:TN],
                             start=True, stop=True, skip_group_check=True)

        wpack_sb = wp.tile([128, WCOLS], FP16, name="wpack_sb")
        nc.sync.dma_start(wpack_sb[:, 0:512], wpackA[:])
        bpack_sb = wp.tile([128, 2], F32, name="bpack_sb")
        nc.sync.dma_start(bpack_sb[:], bpack[:])
        nc.sync.dma_start(wpack_sb[:, 512:WCOLS], wpackB[:])

        def wslab(k):
            return wpack_sb[:, 128 * k:128 * (k + 1)]

        w1z_sb = [wslab(0), wslab(1)]      # z-gate, x rows 0:128 / 128:256
        w1hm2_sb = [wslab(2), wslab(3)]    # -2*w1h
        w2z2_sb = wpack_sb[:, W2OFF:W2OFF + 64]        # 2*w2z
        w2hm4_sb = wpack_sb[:, W2OFF + 64:W2OFF + 128]  # -4*w2h
        w3pair_sb = [wpack_sb[:, W3OFF + 128 * j:W3OFF + 128 * (j + 1)]
                     for j in range(4)]    # 2*w3 doubly embedded per pair

        out_sb = ob.tile([GROUP, TN * N_GRP], F32)

        def x_slice(s):
            col = s * TN
            ci = next(k for k, (c0, cw) in enumerate(chunk_off)
                      if c0 <= col < c0 + cw)
            ensure_chunk(ci)
            ensure_chunk(ci + 1)
            off = col - chunk_off[ci][0]
            cw = chunk_off[ci][1]
            xab = x_tiles[ci]
            return (xab[:, off:off + TN],
                    xab[:, cw + off:cw + off + TN])

        MM = nc.tensor.matmul
        st = {}

        def stage_A(s):
            xa, xb = x_slice(s)
            l1 = pl1.tile([128, 1024], F32, tag="l1p", name=f"l1p{s}")
            MM(l1[:, 0:TN], w1z_sb[0], xa, start=True, stop=False,
               skip_group_check=True)
            MM(l1[:, 0:TN], w1z_sb[1], xb, start=False, stop=True,
               skip_group_check=True)
            MM(l1[:, TN:2 * TN], w1hm2_sb[0], xa, start=True, stop=False,
               skip_group_check=True)
            MM(l1[:, TN:2 * TN], w1hm2_sb[1], xb, start=False, stop=True,
               skip_group_check=True)
            st[("l1", s)] = l1

        def stage_sig1(s):
            l1 = st.pop(("l1", s))
            sv1 = sv1p.tile([128, 1024], SV1DT, tag="sv1", name=f"sv1{s}")
            nc.scalar.activation(sv1[:], l1[:], AF.Sigmoid, scale=-1.0)
            st[("sv1", s)] = sv1

        def stage_h1(s):
            """h1' = (max(v,.5)-.5) * s = h1/2; the 2 is baked into W2."""
            sv1 = st.pop(("sv1", s))
            u1 = u1p.tile([128, TN], SV1DT, tag="u1", name=f"u1{s}")
            nc.vector.tensor_scalar(u1[:], sv1[:, TN:2 * TN], 0.5, 0.5,
                                    op0=OP.max, op1=OP.subtract)
            h1t = h1p.tile([128, TN], FP16, tag="h1t", name=f"h1t{s}")
            nc.vector.tensor_mul(h1t[:], u1[:], sv1[:, 0:TN])
            st[("h1t", s)] = h1t

        def stage_L2(s):
            """L2 matmuls: subtile parity v -> psum partitions 64v:64v+64."""
            h1t = st.pop(("h1t", s))
            m = s // 2
            v = s & 1
            if v == 0:
                st[("zt2", m)] = pzt.tile([128, 1024], F32, tag="zt2",
                                          name=f"zt2{m}")
            zt2 = st[("zt2", m)]
            MM(zt2[64 * v:64 * v + 64, 0:TN], w2z2_sb, h1t[:],
               start=True, stop=True, skip_group_check=True)
            MM(zt2[64 * v:64 * v + 64, TN:2 * TN], w2hm4_sb, h1t[:],
               start=True, stop=True, skip_group_check=True)

        def stage_sig2(m):
            zt2 = st.pop(("zt2", m))
            sv2 = sv2p.tile([128, 1024], SV2DT, tag="sv2", name=f"sv2{m}")
            nc.scalar.activation(sv2[:], zt2[:], AF.Sigmoid, scale=-1.0)
            u2 = u2p.tile([128, TN], SV2DT, tag="u2", name=f"u2{m}")
            nc.vector.tensor_scalar(u2[:], sv2[:, TN:2 * TN], 0.5, 0.5,
                                    op0=OP.max, op1=OP.subtract)
            h2t = h2p.tile([128, TN], FP16, tag="h2t", name=f"h2t{m}")
            nc.vector.tensor_mul(h2t[:], u2[:], sv2[:, 0:TN])
            st[("h2t", m)] = h2t

        def stage_L3(m):
            h2t = st.pop(("h2t", m))
            g = (2 * m) // GROUP
            j = m - g * (GROUP // 2)
            npair = (_gs(g) + 1) // 2
            if j == 0:
                st[("h3p", g)] = ph3.tile([128, 512], F32, tag="h3p",
                                          name=f"h3p{g}")
            MM(st[("h3p", g)][:, 0:TN], w3pair_sb[j], h2t[:], start=(j == 0),
               stop=(j == npair - 1), skip_group_check=True)

        def stage_out(g):
            h3 = st.pop(("h3p", g))
            gs = _gs(g)
            h3s = h3sp.tile([128, TN], FP16, tag="h3s", name=f"h3s{g}")
            nc.vector.tensor_scalar(h3s[0:16 * gs, :], h3[0:16 * gs, 0:TN],
                                    bpack_sb[0:16 * gs, 0:1], 0.0,
                                    op0=OP.add, op1=OP.max)
            opre = po.tile([GROUP, 512], F32, tag="opre", name=f"op{g}")
            MM(opre[0:gs, 0:TN], wpack_sb[0:16 * gs, W4OFF:W4OFF + gs],
               h3s[0:16 * gs, :], start=True, stop=True,
               skip_group_check=True)
            nc.vector.tensor_scalar(out_sb[0:gs, g * TN:(g + 1) * TN],
                                    opre[0:gs, 0:TN], bpack_sb[0:gs, 1:2],
                                    None, op0=OP.add)
            nc.sync.dma_start(out[0:gs, g * TN:(g + 1) * TN],
                              out_sb[0:gs, g * TN:(g + 1) * TN])

        group_after = {}
        for g in range(N_GRP):
            m_last = (g * GROUP + _gs(g) - 1) // 2
            group_after[m_last] = g

        pend_sig2 = []
        sig2_next = []
        pend_l3 = []
        pend_out = []
        for s in range(N_SUB):
            stage_A(s)
            if s >= 1 and (s - 1) % 2 == 1:
                stage_sig1(s - 1)
                stage_h1(s - 1)
                stage_L2(s - 1)   # closes zt2(m): hoisted for early sigma2
                sig2_next.append((s - 1) // 2)
            elif s >= 1:
                stage_sig1(s - 1)
                stage_h1(s - 1)
            if pend_out:
                stage_out(pend_out.pop(0))
            if pend_l3:
                m = pend_l3.pop(0)
                stage_L3(m)
                if m in group_after:
                    pend_out.append(group_after[m])
            if pend_sig2:
                m = pend_sig2.pop(0)
                stage_sig2(m)
                pend_l3.append(m)
            if s >= 1 and (s - 1) % 2 == 0:
                stage_L2(s - 1)
            pend_sig2.extend(sig2_next)
            sig2_next = []
        # drain
        stage_sig1(N_SUB - 1)
        stage_h1(N_SUB - 1)
        pend_sig2.extend(sig2_next)
        sig2_next = []
        while pend_sig2:
            m = pend_sig2.pop(0)
            stage_sig2(m)
            pend_l3.append(m)
        stage_L2(N_SUB - 1)
        if N_SUB % 2 == 1:
            pend_sig2.append((N_SUB - 1) // 2)
        while pend_sig2 or pend_l3 or pend_out:
            if pend_out:
                stage_out(pend_out.pop(0))
            if pend_l3:
                m = pend_l3.pop(0)
                stage_L3(m)
                if m in group_after:
                    pend_out.append(group_after[m])
                continue
            if pend_sig2:
                m = pend_sig2.pop(0)
                stage_sig2(m)
                pend_l3.append(m)

    nc.compile()
    return nc


_NC_CACHE = {}


def _get_nc():
    if "nc" not in _NC_CACHE:
        _NC_CACHE["nc"] = build_nc()
    return _NC_CACHE["nc"]


def make_in_maps(x, w_z1, b_z1, w_r1, b_r1, w_h1, b_h1,
                 w_z2, b_z2, w_r2, b_r2, w_h2, b_h2,
                 w_lin1, b_lin1, w_lin2, b_lin2,
                 n_cores=N_CORES, shard=SHARD):
    f = np.float32
    for b in (b_z1, b_h1, b_z2, b_h2):
        assert not np.any(np.asarray(b)), \
            "sigma-fused gates assume zero gate biases (spec: fill=zeros)"
    w1z = np.asarray((np.asarray(w_z1)[0, 0] + np.asarray(w_z1)[1, 0])[:256], f)
    w1h = np.asarray((np.asarray(w_h1)[0, 0] + np.asarray(w_h1)[1, 0])[:256], f)
    w2z = np.asarray((np.asarray(w_z2)[0, 0] + np.asarray(w_z2)[1, 0])[:128], f)
    w2h = np.asarray((np.asarray(w_h2)[0, 0] + np.asarray(w_h2)[1, 0])[:128], f)
    w3 = np.asarray(w_lin1, f)
    w4 = np.asarray(w_lin2, f)

    wp = np.zeros((128, WCOLS), f)
    wp[:, 0:128] = w1z[0:128]
    wp[:, 128:256] = w1z[128:256]
    wp[:, 256:384] = -2.0 * w1h[0:128]
    wp[:, 384:512] = -2.0 * w1h[128:256]
    wp[:, W2OFF:W2OFF + 64] = 2.0 * w2z          # h1 = 2*h1'
    wp[:, W2OFF + 64:W2OFF + 128] = -4.0 * w2h   # -2 (sigma fuse) * 2
    for j in range(4):
        base = W3OFF + 128 * j
        wp[0:64, base + 32 * j:base + 32 * j + 16] = 2.0 * w3     # h2 = 2*h2'
        wp[64:128, base + 32 * j + 16:base + 32 * j + 32] = 2.0 * w3
    for j in range(GROUP):
        wp[16 * j:16 * j + 16, W4OFF + j] = w4[:, 0]

    bp = np.zeros((128, 2), f)
    bp[:, 0] = np.tile(np.asarray(b_lin1, f), GROUP)
    bp[0:GROUP, 1] = np.asarray(b_lin2, f).reshape(-1)[0]
    wpn = wp.astype(np.float16)
    common = {
        "wpackA": np.ascontiguousarray(wpn[:, 0:512]),
        "wpackB": np.ascontiguousarray(wpn[:, 512:]),
        "bpack": bp,
    }
    x = np.asarray(x, f)
    n = x.shape[0]
    pad = n_cores * shard
    xpad = np.zeros((pad, 256), f)
    xpad[:n] = x
    shards = xpad.reshape(n_cores, shard, 256).transpose(0, 2, 1)  # [c,256,S]
    # chunk-interleaved halves: per chunk [rows 0:128 | rows 128:256]
    maps = []
    for i in range(n_cores):
        xi = shards[i].astype(np.float16)  # [256, SHARD]
        parts = []
        c = 0
        for w in CHUNKS:
            parts.append(xi[0:128, c:c + w])
            parts.append(xi[128:256, c:c + w])
            c += w
        maps.append(dict(common, xt=np.ascontiguousarray(
            np.concatenate(parts, axis=1))))
    return maps


def unscramble(res, n_cores=N_CORES, shard=SHARD):
    full = np.empty(n_cores * shard, np.float32)
    for i in range(n_cores):
        o = res[i]
        for g in range(N_GRP):
            gs = _gs(g)
            for j in range(gs):
                s = g * GROUP + j
                full[i * shard + s * TN:i * shard + (s + 1) * TN] = \
                    o[j, g * TN:(g + 1) * TN]
    return full


def kernel(x, edge_index=None, edge_weight=None,
           w_z1=None, b_z1=None, w_r1=None, b_r1=None, w_h1=None, b_h1=None,
           w_z2=None, b_z2=None, w_r2=None, b_r2=None, w_h2=None, b_h2=None,
           w_lin1=None, b_lin1=None, w_lin2=None, b_lin2=None):
    in_maps = make_in_maps(x, w_z1, b_z1, w_r1, b_r1, w_h1, b_h1,
                           w_z2, b_z2, w_r2, b_r2, w_h2, b_h2,
                           w_lin1, b_lin1, w_lin2, b_lin2)
    nc = _get_nc()
    res = run_bass_kernel_spmd(nc, in_maps, list(range(N_CORES))).results
    n = np.asarray(x).shape[0]
    full = unscramble([res[i]["out"] for i in range(N_CORES)])
    return np.ascontiguousarray(full[:n].reshape(n, 1).astype(np.float32))


# revision 3
# speedup vs baseline: 1.0231x; 1.0038x over previous
"""Trainium2 Bass kernel for nn_EnhancedRecurrentGCN (K=1 DConv DCRNN stack).

Math (h0 == 0 collapses each DCRNN cell; the r-gate is multiplied by zero):
    h1 = relu(sigmoid(-x@W1z) * tanh(x@W1h))     [per node]
    h2 = relu(sigmoid(-h1@W2z) * tanh(h1@W2h))
    y  = relu(h2@W3 + b3) @ W4 + b4

Design (v5, shipped):
 - 12800 nodes/core (padded), TN=512, 25 subtiles.
 - Sigmoid-only activations: tanh(b) = 2*sigmoid(2b)-1, -2 baked into W*h
   slabs; ONE sigmoid ACTIVATE per L1 subtile ([128,1024] psum: z cols
   0:512, -2b cols 512:1024) and one per L2 macro.
 - relu(s*tanh(b)) = s*relu(2v-1) = 2 * (max(v,.5)-.5) * s with the 2
   folded into the next layer's weights.  Per gate-pair: one 4x-mode
   tensor_scalar + one 2x-mode tensor_tensor (bf16/fp16), no STT ops.
 - L2 via [128,64] slabs writing at psum partition base 0/64 (macro pair
   packing) - no zero-embedded slabs, cheaper LDWEIGHTS.
 - x shipped as [128, 2*12800] fp16 (chunk-interleaved halves) so each
   chunk is ONE DMA; first chunk issued before the weight DMAs.
"""

import os
import sys

if "/opt/trn_rl_repo" not in sys.path:
    sys.path.insert(0, "/opt/trn_rl_repo")

from contextlib import ExitStack

import numpy as np

import concourse.mybir as mybir
import concourse.tile as tile
from concourse import bacc
from concourse.bass_utils import run_bass_kernel_spmd

N_CORES = 8
SHARD = 12800
TN = 512
GROUP = 8
N_SUB = SHARD // TN           # 25
N_GRP = (N_SUB + GROUP - 1) // GROUP  # 4 (8+8+8+1)

F32 = mybir.dt.float32
FP16 = mybir.dt.float16
BF16 = mybir.dt.bfloat16
AF = mybir.ActivationFunctionType
OP = mybir.AluOpType

# sigma-output dtypes: bf16 for layer 1 (speed; error averages out through
# the L2 contraction), fp16 for layer 2 (its quantization hits y directly:
# bf16 here costs 1.7e-2 final error vs 6.2e-3 with fp16).
SV1DT = _D = BF16
SV2DT = FP16

# weight pack columns: L1 4x128 | L2 2x64 | L3 4x128 | L4 8
W2OFF = 512
W3OFF = 512 + 128
W4OFF = W3OFF + 512
WCOLS = W4OFF + 8
CHUNKS = [512, 512, 1024, 2048, 2560, 3072, 3072]  # = 12800


def _gs(g):
    return min(GROUP, N_SUB - g * GROUP)


def build_nc():
    nc = bacc.Bacc(None)

    xt = nc.declare_dram_parameter("xt", [128, 2 * SHARD], FP16, isOutput=False)
    wpackA = nc.declare_dram_parameter("wpackA", [128, 512], FP16, isOutput=False)
    wpackB = nc.declare_dram_parameter("wpackB", [128, WCOLS - 512], FP16,
                                       isOutput=False)
    bpack = nc.declare_dram_parameter("bpack", [128, 2], F32, isOutput=False)
    out = nc.declare_dram_parameter("out", [GROUP, TN * N_GRP], F32, isOutput=True)

    with ExitStack() as ctx:
        tc = ctx.enter_context(tile.TileContext(nc, pool_alloc_mode="queue"))
        wp = ctx.enter_context(tc.tile_pool(name="weights", bufs=1))
        xp = ctx.enter_context(tc.tile_pool(name="x", bufs=4))
        sv1p = ctx.enter_context(tc.tile_pool(name="sv1", bufs=3))
        u1p = ctx.enter_context(tc.tile_pool(name="u1", bufs=2))
        h1p = ctx.enter_context(tc.tile_pool(name="h1t", bufs=3))
        sv2p = ctx.enter_context(tc.tile_pool(name="sv2", bufs=2))
        u2p = ctx.enter_context(tc.tile_pool(name="u2", bufs=2))
        h2p = ctx.enter_context(tc.tile_pool(name="h2t", bufs=3))
        h3sp = ctx.enter_context(tc.tile_pool(name="h3s", bufs=2))
        ob = ctx.enter_context(tc.tile_pool(name="outbuf", bufs=1))
        # PSUM: l1p 2x2 banks + zt2 2 + h3p 1 + opre 1 = 8 banks
        pl1 = ctx.enter_context(tc.tile_pool(name="pl1", bufs=2, space="PSUM"))
        pzt = ctx.enter_context(tc.tile_pool(name="pzt", bufs=1, space="PSUM"))
        ph3 = ctx.enter_context(tc.tile_pool(name="ph3", bufs=1, space="PSUM"))
        po = ctx.enter_context(tc.tile_pool(name="po", bufs=1, space="PSUM"))

        x_tiles = {}
        chunk_off = []
        c = 0
        for w in CHUNKS:
            chunk_off.append((c, w))
            c += w

        def ensure_chunk(ci, eng=None):
            if ci in x_tiles or ci >= len(chunk_off):
                return
            c0, cw = chunk_off[ci]
            xab = xp.tile([128, 2 * cw], FP16, tag="xab", name=f"xab{ci}")
            (eng or nc.sync).dma_start(xab[:], xt[:, 2 * c0:2 * c0 + 2 * cw])
            x_tiles[ci] = xab

        # scalar-queue HWDGE: its preamble drains ~2us before sync's, and the
        # ACT queue is otherwise idle until the first sigmoid at ~12us.
        ensure_chunk(0, eng=nc.scalar)

        # PE warm-up on junk data: keeps the HAM activity window hot while
        # the first x chunk is in flight; psum bank is overwritten later.
        junk = wp.tile([128, 512], FP16, name="junk")
        nc.gpsimd.memset(junk[:], 0.0)
        warm_ps = ph3.tile([128, 512], F32, tag="h3p", name="warm")
        for _ in range(10):
            nc.tensor.matmul(warm_ps[:, 0:TN], junk[:, 0:128], junk[:, 0:TN],
                             start=True, stop=True, skip_group_check=True)

        wpack_sb = wp.tile([128, WCOLS], FP16, name="wpack_sb")
        nc.scalar.dma_start(wpack_sb[:, 0:512], wpackA[:])
        bpack_sb = wp.tile([128, 2], F32, name="bpack_sb")
        nc.sync.dma_start(bpack_sb[:], bpack[:])
        nc.sync.dma_start(wpack_sb[:, 512:WCOLS], wpackB[:])

        def wslab(k):
            return wpack_sb[:, 128 * k:128 * (k + 1)]

        w1z_sb = [wslab(0), wslab(1)]      # z-gate, x rows 0:128 / 128:256
        w1hm2_sb = [wslab(2), wslab(3)]    # -2*w1h
        w2z2_sb = wpack_sb[:, W2OFF:W2OFF + 64]        # 2*w2z
        w2hm4_sb = wpack_sb[:, W2OFF + 64:W2OFF + 128]  # -4*w2h
        w3pair_sb = [wpack_sb[:, W3OFF + 128 * j:W3OFF + 128 * (j + 1)]
                     for j in range(4)]    # 2*w3 doubly embedded per pair

        out_sb = ob.tile([GROUP, TN * N_GRP], F32)

        def x_slice(s):
            col = s * TN
            ci = next(k for k, (c0, cw) in enumerate(chunk_off)
                      if c0 <= col < c0 + cw)
            ensure_chunk(ci)
            ensure_chunk(ci + 1)
            off = col - chunk_off[ci][0]
            cw = chunk_off[ci][1]
            xab = x_tiles[ci]
            return (xab[:, off:off + TN],
                    xab[:, cw + off:cw + off + TN])

        MM = nc.tensor.matmul
        st = {}

        def stage_A(s):
            xa, xb = x_slice(s)
            l1 = pl1.tile([128, 1024], F32, tag="l1p", name=f"l1p{s}")
            MM(l1[:, 0:TN], w1z_sb[0], xa, start=True, stop=False,
               skip_group_check=True)
            MM(l1[:, 0:TN], w1z_sb[1], xb, start=False, stop=True,
               skip_group_check=True)
            MM(l1[:, TN:2 * TN], w1hm2_sb[0], xa, start=True, stop=False,
               skip_group_check=True)
            MM(l1[:, TN:2 * TN], w1hm2_sb[1], xb, start=False, stop=True,
               skip_group_check=True)
            st[("l1", s)] = l1

        def stage_sig1(s):
            l1 = st.pop(("l1", s))
            sv1 = sv1p.tile([128, 1024], SV1DT, tag="sv1", name=f"sv1{s}")
            nc.scalar.activation(sv1[:], l1[:], AF.Sigmoid, scale=-1.0)
            st[("sv1", s)] = sv1

        def stage_h1(s):
            """h1' = (max(v,.5)-.5) * s = h1/2; the 2 is baked into W2."""
            sv1 = st.pop(("sv1", s))
            u1 = u1p.tile([128, TN], SV1DT, tag="u1", name=f"u1{s}")
            nc.vector.tensor_scalar(u1[:], sv1[:, TN:2 * TN], 0.5, 0.5,
                                    op0=OP.max, op1=OP.subtract)
            h1t = h1p.tile([128, TN], FP16, tag="h1t", name=f"h1t{s}")
            nc.vector.tensor_mul(h1t[:], u1[:], sv1[:, 0:TN])
            st[("h1t", s)] = h1t

        def stage_L2(s):
            """L2 matmuls: subtile parity v -> psum partitions 64v:64v+64."""
            h1t = st.pop(("h1t", s))
            m = s // 2
            v = s & 1
            if v == 0:
                st[("zt2", m)] = pzt.tile([128, 1024], F32, tag="zt2",
                                          name=f"zt2{m}")
            zt2 = st[("zt2", m)]
            MM(zt2[64 * v:64 * v + 64, 0:TN], w2z2_sb, h1t[:],
               start=True, stop=True, skip_group_check=True)
            MM(zt2[64 * v:64 * v + 64, TN:2 * TN], w2hm4_sb, h1t[:],
               start=True, stop=True, skip_group_check=True)

        def stage_sig2(m):
            zt2 = st.pop(("zt2", m))
            sv2 = sv2p.tile([128, 1024], SV2DT, tag="sv2", name=f"sv2{m}")
            nc.scalar.activation(sv2[:], zt2[:], AF.Sigmoid, scale=-1.0)
            u2 = u2p.tile([128, TN], SV2DT, tag="u2", name=f"u2{m}")
            nc.vector.tensor_scalar(u2[:], sv2[:, TN:2 * TN], 0.5, 0.5,
                                    op0=OP.max, op1=OP.subtract)
            h2t = h2p.tile([128, TN], FP16, tag="h2t", name=f"h2t{m}")
            nc.vector.tensor_mul(h2t[:], u2[:], sv2[:, 0:TN])
            st[("h2t", m)] = h2t

        def stage_L3(m):
            h2t = st.pop(("h2t", m))
            g = (2 * m) // GROUP
            j = m - g * (GROUP // 2)
            npair = (_gs(g) + 1) // 2
            if j == 0:
                st[("h3p", g)] = ph3.tile([128, 512], F32, tag="h3p",
                                          name=f"h3p{g}")
            MM(st[("h3p", g)][:, 0:TN], w3pair_sb[j], h2t[:], start=(j == 0),
               stop=(j == npair - 1), skip_group_check=True)

        def stage_out(g):
            h3 = st.pop(("h3p", g))
            gs = _gs(g)
            h3s = h3sp.tile([128, TN], FP16, tag="h3s", name=f"h3s{g}")
            nc.vector.tensor_scalar(h3s[0:16 * gs, :], h3[0:16 * gs, 0:TN],
                                    bpack_sb[0:16 * gs, 0:1], 0.0,
                                    op0=OP.add, op1=OP.max)
            opre = po.tile([GROUP, 512], F32, tag="opre", name=f"op{g}")
            MM(opre[0:gs, 0:TN], wpack_sb[0:16 * gs, W4OFF:W4OFF + gs],
               h3s[0:16 * gs, :], start=True, stop=True,
               skip_group_check=True)
            nc.vector.tensor_scalar(out_sb[0:gs, g * TN:(g + 1) * TN],
                                    opre[0:gs, 0:TN], bpack_sb[0:gs, 1:2],
                                    None, op0=OP.add)
            nc.sync.dma_start(out[0:gs, g * TN:(g + 1) * TN],
                              out_sb[0:gs, g * TN:(g + 1) * TN])

        group_after = {}
        for g in range(N_GRP):
            m_last = (g * GROUP + _gs(g) - 1) // 2
            group_after[m_last] = g

        pend_sig2 = []
        sig2_next = []
        pend_l3 = []
        pend_out = []
        for s in range(N_SUB):
            stage_A(s)
            if s >= 1 and (s - 1) % 2 == 1:
                stage_sig1(s - 1)
                stage_h1(s - 1)
                stage_L2(s - 1)   # closes zt2(m): hoisted for early sigma2
                sig2_next.append((s - 1) // 2)
            elif s >= 1:
                stage_sig1(s - 1)
                stage_h1(s - 1)
            if pend_out:
                stage_out(pend_out.pop(0))
            if pend_l3:
                m = pend_l3.pop(0)
                stage_L3(m)
                if m in group_after:
                    pend_out.append(group_after[m])
            if pend_sig2:
                m = pend_sig2.pop(0)
                stage_sig2(m)
                pend_l3.append(m)
            if s >= 1 and (s - 1) % 2 == 0:
                stage_L2(s - 1)
            pend_sig2.extend(sig2_next)
            sig2_next = []
        # drain
        stage_sig1(N_SUB - 1)
        stage_h1(N_SUB - 1)
        pend_sig2.extend(sig2_next)
        sig2_next = []
        while pend_sig2:
            m = pend_sig2.pop(0)
            stage_sig2(m)
            pend_l3.append(m)
        stage_L2(N_SUB - 1)
        if N_SUB % 2 == 1:
            pend_sig2.append((N_SUB - 1) // 2)
        while pend_sig2 or pend_l3 or pend_out:
            if pend_out:
                stage_out(pend_out.pop(0))
            if pend_l3:
                m = pend_l3.pop(0)
                stage_L3(m)
                if m in group_after:
                    pend_out.append(group_after[m])
                continue
            if pend_sig2:
                m = pend_sig2.pop(0)
                stage_sig2(m)
                pend_l3.append(m)

    nc.compile()
    return nc


_NC_CACHE = {}


def _get_nc():
    if "nc" not in _NC_CACHE:
        _NC_CACHE["nc"] = build_nc()
    return _NC_CACHE["nc"]


def make_in_maps(x, w_z1, b_z1, w_r1, b_r1, w_h1, b_h1,
                 w_z2, b_z2, w_r2, b_r2, w_h2, b_h2,
                 w_lin1, b_lin1, w_lin2, b_lin2,
                 n_cores=N_CORES, shard=SHARD):
    f = np.float32
    for b in (b_z1, b_h1, b_z2, b_h2):
        assert not np.any(np.asarray(b)), \
            "sigma-fused gates assume zero gate biases (spec: fill=zeros)"
    w1z = np.asarray((np.asarray(w_z1)[0, 0] + np.asarray(w_z1)[1, 0])[:256], f)
    w1h = np.asarray((np.asarray(w_h1)[0, 0] + np.asarray(w_h1)[1, 0])[:256], f)
    w2z = np.asarray((np.asarray(w_z2)[0, 0] + np.asarray(w_z2)[1, 0])[:128], f)
    w2h = np.asarray((np.asarray(w_h2)[0, 0] + np.asarray(w_h2)[1, 0])[:128], f)
    w3 = np.asarray(w_lin1, f)
    w4 = np.asarray(w_lin2, f)

    wp = np.zeros((128, WCOLS), f)
    wp[:, 0:128] = w1z[0:128]
    wp[:, 128:256] = w1z[128:256]
    wp[:, 256:384] = -2.0 * w1h[0:128]
    wp[:, 384:512] = -2.0 * w1h[128:256]
    wp[:, W2OFF:W2OFF + 64] = 2.0 * w2z          # h1 = 2*h1'
    wp[:, W2OFF + 64:W2OFF + 128] = -4.0 * w2h   # -2 (sigma fuse) * 2
    for j in range(4):
        base = W3OFF + 128 * j
        wp[0:64, base + 32 * j:base + 32 * j + 16] = 2.0 * w3     # h2 = 2*h2'
        wp[64:128, base + 32 * j + 16:base + 32 * j + 32] = 2.0 * w3
    for j in range(GROUP):
        wp[16 * j:16 * j + 16, W4OFF + j] = w4[:, 0]

    bp = np.zeros((128, 2), f)
    bp[:, 0] = np.tile(np.asarray(b_lin1, f), GROUP)
    bp[0:GROUP, 1] = np.asarray(b_lin2, f).reshape(-1)[0]
    wpn = wp.astype(np.float16)
    common = {
        "wpackA": np.ascontiguousarray(wpn[:, 0:512]),
        "wpackB": np.ascontiguousarray(wpn[:, 512:]),
        "bpack": bp,
    }
    x = np.asarray(x, f)
    n = x.shape[0]
    pad = n_cores * shard
    xpad = np.zeros((pad, 256), f)
    xpad[:n] = x
    shards = xpad.reshape(n_cores, shard, 256).transpose(0, 2, 1)  # [c,256,S]
    # chunk-interleaved halves: per chunk [rows 0:128 | rows 128:256]
    maps = []
    for i in range(n_cores):
        xi = shards[i].astype(np.float16)  # [256, SHARD]
        parts = []
        c = 0
        for w in CHUNKS:
            parts.append(xi[0:128, c:c + w])
            parts.append(xi[128:256, c:c + w])
            c += w
        maps.append(dict(common, xt=np.ascontiguousarray(
            np.concatenate(parts, axis=1))))
    return maps


def unscramble(res, n_cores=N_CORES, shard=SHARD):
    full = np.empty(n_cores * shard, np.float32)
    for i in range(n_cores):
        o = res[i]
        for g in range(N_GRP):
            gs = _gs(g)
            for j in range(gs):
                s = g * GROUP + j
                full[i * shard + s * TN:i * shard + (s + 1) * TN] = \
                    o[j, g * TN:(g + 1) * TN]
    return full


def kernel(x, edge_index=None, edge_weight=None,
           w_z1=None, b_z1=None, w_r1=None, b_r1=None, w_h1=None, b_h1=None,
           w_z2=None, b_z2=None, w_r2=None, b_r2=None, w_h2=None, b_h2=None,
           w_lin1=None, b_lin1=None, w_lin2=None, b_lin2=None):
    in_maps = make_in_maps(x, w_z1, b_z1, w_r1, b_r1, w_h1, b_h1,
                           w_z2, b_z2, w_r2, b_r2, w_h2, b_h2,
                           w_lin1, b_lin1, w_lin2, b_lin2)
    nc = _get_nc()
    res = run_bass_kernel_spmd(nc, in_maps, list(range(N_CORES))).results
    n = np.asarray(x).shape[0]
    full = unscramble([res[i]["out"] for i in range(N_CORES)])
    return np.ascontiguousarray(full[:n].reshape(n, 1).astype(np.float32))
